# revision 1
# baseline (speedup 1.0000x reference)
"""Expert-parallel SwiGLU MLP (MoE experts) for 8 Trainium2 NeuronCores.

Problem: routed_in_egD [E*G, D] fp32, w1/w3 [E, D, F], w2 [E, F, D], E=8,
G=2048, D=2048, F=5632.  reference:
    x_egD = routed.reshape(E, G, D)
    mid   = silu(x @ w1) * (x @ w3)          # [E, G, F]
    out   = (mid @ w2).reshape(E*G, D)

Sharding: expert-parallel — core e gets expert e's x slice + weights; no
collectives.  Each core runs three 2048x2048x5632-class GEMMs (~142 GFLOP).

Per-core kernel (stage-1 matmuls fp32r, stage-2 bf16, PSUM fp32):
  phase 0: PE-transpose x [G,D] -> xT [D,G] resident in SBUF (fp32r).
  phase 1: per f-chunk (128 rows of F), per g-half: gateT/upT = w1/w3.T @ x
           accumulated over D in PSUM; SwiGLU (ACT silu -> bf16, DVE mul
           in place); midT [F,G] spilled to DRAM as bf16.
  phase 2: out[g,d] = sum_f midT[f,g]*w2[f,d]: mid panels stationary (bf16),
           w2 panels DMA-cast fp32->bf16 (moving), PSUM accumulation over F.
           Output lands in natural [G, D] layout.
"""

import numpy as np

import concourse.mybir as mybir
import concourse.tile as tile
from concourse import bacc
from concourse.bass_utils import run_bass_kernel_spmd
from concourse.masks import make_identity

E, G, D, F = 8, 2048, 2048, 5632
P = 128
DO = D // P      # 16 d-chunks
FC = F // P      # 44 f-chunks
GO = G // P      # 16 g-chunks

F32 = mybir.dt.float32
F32R = mybir.dt.float32r
BF16 = mybir.dt.bfloat16


def build_nc():
    nc = bacc.Bacc("TRN2", target_bir_lowering=False)
    x = nc.dram_tensor("x", [G, D], F32, kind="ExternalInput").ap()
    w1 = nc.dram_tensor("w1", [D, F], F32, kind="ExternalInput").ap()
    w2 = nc.dram_tensor("w2", [F, D], F32, kind="ExternalInput").ap()
    w3 = nc.dram_tensor("w3", [D, F], F32, kind="ExternalInput").ap()
    out = nc.dram_tensor("out", [G, D], F32, kind="ExternalOutput").ap()

    w1r = w1.rearrange("(do p) f -> p do f", p=P)
    w3r = w3.rearrange("(do p) f -> p do f", p=P)
    w2r = w2.rearrange("(fo p) d -> p fo d", p=P)

    with tile.TileContext(nc) as tc:
        dram = tc.alloc_tile_pool(name="dram", bufs=1, space="DRAM")
        # midT stored gp-blocked and f-major within each g-panel:
        # mid5[p, gp, fo, g'] = silu/up product for f = fo*128+p, g = gp*256+g'.
        # Phase-1 writes are per-partition contiguous 512B; phase-2 panel reads
        # are per-partition contiguous 22KB (128 DMA blocks instead of 5632).
        mid5 = dram.tile([P, 8, FC, 256], BF16)

        wp = tc.alloc_tile_pool(name="wp", bufs=3)
        mp = tc.alloc_tile_pool(name="mp", bufs=3)
        xtp = tc.alloc_tile_pool(name="xtp", bufs=1)
        xT = xtp.tile([P, DO, G], F32R)

        # ---- phase 0: x [G, D] -> xT [d_in, d_out, g] (fp32r)
        p0 = tc.alloc_tile_pool(name="p0", bufs=8)
        idp = tc.alloc_tile_pool(name="idp", bufs=1)
        p0ps = tc.alloc_tile_pool(name="p0ps", bufs=6, space="PSUM")
        ident = idp.tile([P, P], F32)
        make_identity(nc, ident)
        for go in range(GO):
            for q in range(4):
                xsq = p0.tile([P, 512], F32, tag="xs")
                nc.sync.dma_start(
                    xsq, x[go * P : (go + 1) * P, q * 512 : (q + 1) * 512]
                )
                tp = p0ps.tile([P, 4, P], F32, tag="tp")
                for j in range(4):
                    nc.tensor.transpose(tp[:, j], xsq[:, j * P : (j + 1) * P], ident)
                nc.vector.tensor_copy(
                    xT[:, q * 4 : (q + 1) * 4, go * P : (go + 1) * P], tp
                )
        idp.release()
        p0.release()
        p0ps.release()

        # ---- phase 1: midT[f, g] = silu(w1.T x) * (w3.T x), spill bf16
        ps1g = tc.alloc_tile_pool(name="ps1g", bufs=2, space="PSUM")
        ps1u = tc.alloc_tile_pool(name="ps1u", bufs=2, space="PSUM")
        for fc in range(FC):
            w1t = wp.tile([P, DO, P], F32R, tag="w1")
            nc.gpsimd.dma_start(w1t, w1r[:, :, fc * P : (fc + 1) * P])
            w3t = wp.tile([P, DO, P], F32R, tag="w3")
            nc.gpsimd.dma_start(w3t, w3r[:, :, fc * P : (fc + 1) * P])
            for gh in range(2):
                pg = ps1g.tile([P, 2, 512], F32, tag="pg")
                pu = ps1u.tile([P, 2, 512], F32, tag="pu")
                for d in range(DO):
                    st, sp_ = (d == 0), (d == DO - 1)
                    for j in range(2):
                        gsl = slice((gh * 2 + j) * 512, (gh * 2 + j + 1) * 512)
                        nc.tensor.matmul(
                            pg[:, j], w1t[:, d], xT[:, d, gsl], start=st, stop=sp_
                        )
                        nc.tensor.matmul(
                            pu[:, j], w3t[:, d], xT[:, d, gsl], start=st, stop=sp_
                        )
                mo = mp.tile([P, 4, 256], BF16, tag="mo")
                nc.scalar.activation(
                    mo, pg.rearrange("p j g -> p (j g)"),
                    mybir.ActivationFunctionType.Silu,
                )
                nc.vector.tensor_mul(mo, mo, pu.rearrange("p j g -> p (j g)"))
                for j in range(4):
                    nc.scalar.dma_start(mid5[:, gh * 4 + j, fc], mo[:, j])
        xtp.release()
        mp.release()
        wp.release()
        ps1u.release()
        ps1g.release()

        # ---- phase 2: out[g, d] = midT.T @ w2 (bf16 x bf16, fp32 psum)
        w2p = tc.alloc_tile_pool(name="w2p", bufs=2, side="right")
        mqp = tc.alloc_tile_pool(name="mqp", bufs=3, side="right")
        op = tc.alloc_tile_pool(name="op", bufs=6, side="right")
        ps2 = tc.alloc_tile_pool(name="ps2", bufs=3, space="PSUM")
        w2bounds = [0, 6, 12, 18, 24, 29, 34, 39, 44]
        for dq in range(4):
            w2q = w2p.tile([P, FC, 512], BF16, tag="w2q")
            if dq == 0:
                # chunk by fo so the first accumulation steps can start early
                for k in range(8):
                    lo, hi = w2bounds[k], w2bounds[k + 1]
                    nc.gpsimd.dma_start(
                        w2q[:, lo:hi, :], w2r[:, lo:hi, 0:512]
                    )
            else:
                nc.gpsimd.dma_start(w2q, w2r[:, :, dq * 512 : (dq + 1) * 512])
            for gp in range(8):
                mq = mqp.tile([P, FC, 256], BF16, tag="mq")
                # alternate HWDGE queues so consecutive panel loads overlap
                dma_eng = nc.sync if gp % 2 == 0 else nc.scalar
                if dq == 0 and gp == 0:
                    for k in range(8):
                        lo, hi = w2bounds[k], w2bounds[k + 1]
                        nc.sync.dma_start(
                            mq[:, lo:hi, :], mid5[:, 0, lo:hi, :]
                        )
                else:
                    dma_eng.dma_start(mq, mid5[:, gp])
                po = ps2.tile([P, 2, 512], F32, tag="po")
                for fo in range(FC):
                    st, sp_ = (fo == 0), (fo == FC - 1)
                    for gc in range(2):
                        nc.tensor.matmul(
                            po[:, gc],
                            mq[:, fo, gc * P : (gc + 1) * P],
                            w2q[:, fo],
                            start=st,
                            stop=sp_,
                        )
                for gc in range(2):
                    ot = op.tile([P, 512], F32, tag="ot")
                    nc.vector.tensor_copy(ot, po[:, gc])
                    g0 = (gp * 2 + gc) * P
                    nc.scalar.dma_start(
                        out[g0 : g0 + P, dq * 512 : (dq + 1) * 512], ot
                    )
        op.release()
        mqp.release()
        w2p.release()
        ps2.release()
        dram.release()
    nc.compile()
    return nc


_NC_CACHE = None


def _get_nc():
    global _NC_CACHE
    if _NC_CACHE is None:
        _NC_CACHE = build_nc()
    return _NC_CACHE


def _in_maps(routed_in_egD, w1, w2, w3):
    x = np.ascontiguousarray(np.asarray(routed_in_egD, dtype=np.float32))
    w1 = np.ascontiguousarray(np.asarray(w1, dtype=np.float32))
    w2 = np.ascontiguousarray(np.asarray(w2, dtype=np.float32))
    w3 = np.ascontiguousarray(np.asarray(w3, dtype=np.float32))
    x_e = x.reshape(E, G, D)
    return [
        {"x": x_e[e], "w1": w1[e], "w2": w2[e], "w3": w3[e]} for e in range(E)
    ]


def kernel(routed_in_egD, w1, w2, w3):
    nc = _get_nc()
    in_maps = _in_maps(routed_in_egD, w1, w2, w3)
    try:
        res = run_bass_kernel_spmd(nc, in_maps, core_ids=list(range(E)))
    except Exception:
        # the first execute after process start occasionally dies with a
        # transient NRT_EXEC_UNIT_UNRECOVERABLE through the PJRT tunnel;
        # a straight retry has always succeeded
        res = run_bass_kernel_spmd(nc, in_maps, core_ids=list(range(E)))
    return np.concatenate([r["out"] for r in res.results], axis=0)


def run_traced(routed_in_egD, w1, w2, w3, **trace_kwargs):
    """For test.py: run with NTFF tracing; returns (full_out, BassKernelResults)."""
    nc = _get_nc()
    res = run_bass_kernel_spmd(
        nc,
        _in_maps(routed_in_egD, w1, w2, w3),
        core_ids=list(range(E)),
        trace=True,
        **trace_kwargs,
    )
    out = np.concatenate([r["out"] for r in res.results], axis=0)
    return out, res



# revision 6
# speedup vs baseline: 1.0201x; 1.0201x over previous
"""Expert-parallel SwiGLU MLP (MoE experts) for 8 Trainium2 NeuronCores.

Problem: routed_in_egD [E*G, D] fp32, w1/w3 [E, D, F], w2 [E, F, D], E=8,
G=2048, D=2048, F=5632.  reference:
    x_egD = routed.reshape(E, G, D)
    mid   = silu(x @ w1) * (x @ w3)          # [E, G, F]
    out   = (mid @ w2).reshape(E*G, D)

Sharding: expert-parallel - core e gets expert e's x slice + weights; no
collectives.  Each core runs three 2048x2048x5632-class GEMMs (~142 GFLOP),
matmul-roofline-bound at the bf16 PE rate (1 col/cycle @ 2.4 GHz -> ~1.80 ms
of pure matmul streaming per core).

Per-core kernel (all matmuls bf16, PSUM fp32):
  phase A: x [G,D] fp32 -> HWDGE load -> DVE cast bf16 -> xbar DMA-transpose
           (SBUF->SBUF, contiguous temp) -> DVE copy into xT slab [P,DO,G].
           d-index mapping d = p*16 + do (xbar row order); w1/w3 tiles use the
           matching rearrange so contraction pairs line up.  No PE work at all,
           so the PE spends the whole kernel on the real GEMMs.
  phase 1: gh-outer (g-halves) so xT for gh=0 is ready after 1/2 the x load,
           and mid panels for gp 0-3 are complete at the phase midpoint
           (lets phase-2 mq prefetch cross the boundary).  Per (gh, fc):
           gateT/upT = w1/w3.T @ x accumulated over d in PSUM; SwiGLU
           (ACT silu -> bf16, DVE mul); midT spilled to DRAM bf16 per-gp
           tiles (separate tiles => precise DMA deps for early prefetch).
  phase 2: out[g,d] = sum_f midT[f,g]*w2[f,d]: mid panels stationary (bf16),
           w2 panels DMA-cast fp32->bf16 (moving), PSUM accumulation over F.
           mqp pool is pre-allocated (right side) so gp 0-2 panel loads run
           during the phase-1 tail; w2q dq=0 load is chunked so the first
           accumulation steps start ~2 us after the last phase-1 matmul.
"""

import numpy as np

import concourse.mybir as mybir
import concourse.tile as tile
from concourse import bacc
from concourse.bass_utils import run_bass_kernel_spmd

E, G, D, F = 8, 2048, 2048, 5632
P = 128
DO = D // P      # 16 d-chunks (contraction steps)
FC = F // P      # 44 f-chunks
GO = G // P      # 16 g-chunks

F32 = mybir.dt.float32
BF16 = mybir.dt.bfloat16


def build_nc():
    nc = bacc.Bacc("TRN2", target_bir_lowering=False)
    x = nc.dram_tensor("x", [G, D], F32, kind="ExternalInput").ap()
    w1 = nc.dram_tensor("w1", [D, F], F32, kind="ExternalInput").ap()
    w2 = nc.dram_tensor("w2", [F, D], F32, kind="ExternalInput").ap()
    w3 = nc.dram_tensor("w3", [D, F], F32, kind="ExternalInput").ap()
    out = nc.dram_tensor("out", [G, D], F32, kind="ExternalOutput").ap()

    # xbar transpose of x[g-chunk, :] into xtt [p, do, g] maps rows as
    # d = do*128 + p, i.e. the natural d-chunking: partition p within chunk do.
    w1r = w1.rearrange("(do p) f -> p do f", p=P)
    w3r = w3.rearrange("(do p) f -> p do f", p=P)
    w2r = w2.rearrange("(fo p) d -> p fo d", p=P)

    with tile.TileContext(nc) as tc:
        dram = tc.alloc_tile_pool(name="dram", bufs=1, space="DRAM")
        # per-gp mid tiles: mids[gp][p, fo, g'] = mid for f = fo*128+p,
        # g = gp*256 + g'.  Phase-1 writes are per-partition contiguous 512B;
        # phase-2 panel reads are per-partition contiguous 22KB.
        mids = [
            dram.tile([P, FC, 256], BF16, tag=f"mid{gp}", name=f"mid{gp}")
            for gp in range(8)
        ]

        # mqp pre-allocated on the right so its space never overlaps the
        # phase-1 pools: mq panel loads can run during the phase-1 tail.
        mqp = tc.alloc_tile_pool(name="mqp", bufs=3, side="right")

        xtp = tc.alloc_tile_pool(name="xtp", bufs=1)
        xT = xtp.tile([P, DO, G], BF16)

        # ---- phase A: x -> bf16 -> xT [d_in, d_out, g] via xbar transpose
        xfp = tc.alloc_tile_pool(name="xfp", bufs=3)
        xcp = tc.alloc_tile_pool(name="xcp", bufs=2)
        xttp = tc.alloc_tile_pool(name="xttp", bufs=3)
        for q in range(GO):
            xf = xfp.tile([P, D], F32, tag="xf")
            nc.sync.dma_start(xf, x[q * P : (q + 1) * P, :])
            xc = xcp.tile([P, D], BF16, tag="xc")
            # cast on ACT so the DVE queue holds only the xtt->xT copies:
            # a copy stalled on the xbar transpose must not block later casts
            nc.scalar.activation(xc, xf, mybir.ActivationFunctionType.Copy)
            xtt = xttp.tile([P, DO, P], BF16, tag="xtt")
            nc.scalar.dma_start_transpose(xtt, xc)
            nc.vector.tensor_copy(xT[:, :, q * P : (q + 1) * P], xtt)

        # ---- phase 1: midT[f, g] = silu(w1.T x) * (w3.T x), spill bf16
        wp = tc.alloc_tile_pool(name="wp", bufs=3)
        mp = tc.alloc_tile_pool(name="mp", bufs=4)
        ps1g = tc.alloc_tile_pool(name="ps1g", bufs=2, space="PSUM")
        ps1u = tc.alloc_tile_pool(name="ps1u", bufs=2, space="PSUM")
        for gh in range(2):
            g0 = gh * 1024
            for fc in range(FC):
                w1t = wp.tile([P, DO, P], BF16, tag="w1")
                nc.gpsimd.dma_start(w1t, w1r[:, :, fc * P : (fc + 1) * P])
                w3t = wp.tile([P, DO, P], BF16, tag="w3")
                nc.gpsimd.dma_start(w3t, w3r[:, :, fc * P : (fc + 1) * P])
                pg = ps1g.tile([P, 2, 512], F32, tag="pg")
                pu = ps1u.tile([P, 2, 512], F32, tag="pu")
                # j-outer: the first 32 matmuls only touch xT g-cols [g0, g0+512)
                # (x-chunks q0-q3 for gh=0), so the PE can start ~4 chunks into
                # the phase-A supply instead of waiting for 8.
                for j in range(2):
                    gsl = slice(g0 + j * 512, g0 + (j + 1) * 512)
                    for d in range(DO):
                        st, sp_ = (d == 0), (d == DO - 1)
                        nc.tensor.matmul(
                            pg[:, j], w1t[:, d], xT[:, d, gsl],
                            start=st, stop=sp_,
                        )
                        nc.tensor.matmul(
                            pu[:, j], w3t[:, d], xT[:, d, gsl],
                            start=st, stop=sp_,
                        )
                mo = mp.tile([P, 4, 256], BF16, tag="mo")
                nc.scalar.activation(
                    mo, pg.rearrange("p j g -> p (j g)"),
                    mybir.ActivationFunctionType.Silu,
                )
                nc.vector.tensor_mul(mo, mo, pu.rearrange("p j g -> p (j g)"))
                for j in range(4):
                    nc.scalar.dma_start(mids[gh * 4 + j][:, fc], mo[:, j])
        mp.release()
        wp.release()
        xttp.release()
        xcp.release()
        xfp.release()
        xtp.release()
        ps1u.release()
        ps1g.release()

        # ---- phase 2: out[g, d] = midT.T @ w2 (bf16 x bf16, fp32 psum)
        w2p = tc.alloc_tile_pool(name="w2p", bufs=2, side="right")
        op = tc.alloc_tile_pool(name="op", bufs=6, side="right")
        ps2 = tc.alloc_tile_pool(name="ps2", bufs=3, space="PSUM")
        w2bounds = [0, 3, 6, 12, 18, 24, 29, 34, 39, 44]
        for dq in range(4):
            w2q = w2p.tile([P, FC, 512], BF16, tag="w2q")
            if dq == 0:
                # chunk by fo so the first accumulation steps start early
                for k in range(len(w2bounds) - 1):
                    lo, hi = w2bounds[k], w2bounds[k + 1]
                    nc.gpsimd.dma_start(
                        w2q[:, lo:hi, :], w2r[:, lo:hi, 0:512]
                    )
            else:
                nc.gpsimd.dma_start(w2q, w2r[:, :, dq * 512 : (dq + 1) * 512])
            for gp in range(8):
                mq = mqp.tile([P, FC, 256], BF16, tag="mq")
                dma_eng = nc.sync if gp % 2 == 0 else nc.scalar
                dma_eng.dma_start(mq, mids[gp])
                po = ps2.tile([P, 2, 512], F32, tag="po")
                for fo in range(FC):
                    st, sp_ = (fo == 0), (fo == FC - 1)
                    for gc in range(2):
                        nc.tensor.matmul(
                            po[:, gc],
                            mq[:, fo, gc * P : (gc + 1) * P],
                            w2q[:, fo],
                            start=st,
                            stop=sp_,
                        )
                for gc in range(2):
                    ot = op.tile([P, 512], F32, tag="ot")
                    nc.vector.tensor_copy(ot, po[:, gc])
                    g00 = (gp * 2 + gc) * P
                    nc.sync.dma_start(
                        out[g00 : g00 + P, dq * 512 : (dq + 1) * 512], ot
                    )
        op.release()
        w2p.release()
        ps2.release()
        mqp.release()
        dram.release()
    nc.compile()
    return nc


_NC_CACHE = None


def _get_nc():
    global _NC_CACHE
    if _NC_CACHE is None:
        _NC_CACHE = build_nc()
    return _NC_CACHE


def _in_maps(routed_in_egD, w1, w2, w3):
    x = np.ascontiguousarray(np.asarray(routed_in_egD, dtype=np.float32))
    w1 = np.ascontiguousarray(np.asarray(w1, dtype=np.float32))
    w2 = np.ascontiguousarray(np.asarray(w2, dtype=np.float32))
    w3 = np.ascontiguousarray(np.asarray(w3, dtype=np.float32))
    x_e = x.reshape(E, G, D)
    return [
        {"x": x_e[e], "w1": w1[e], "w2": w2[e], "w3": w3[e]} for e in range(E)
    ]


def kernel(routed_in_egD, w1, w2, w3):
    nc = _get_nc()
    in_maps = _in_maps(routed_in_egD, w1, w2, w3)
    try:
        res = run_bass_kernel_spmd(nc, in_maps, core_ids=list(range(E)))
    except Exception:
        # the first execute after process start occasionally dies with a
        # transient NRT_EXEC_UNIT_UNRECOVERABLE through the PJRT tunnel;
        # a straight retry has always succeeded
        res = run_bass_kernel_spmd(nc, in_maps, core_ids=list(range(E)))
    return np.concatenate([r["out"] for r in res.results], axis=0)


def run_traced(routed_in_egD, w1, w2, w3, **trace_kwargs):
    """For test.py: run with NTFF tracing; returns (full_out, BassKernelResults)."""
    nc = _get_nc()
    res = run_bass_kernel_spmd(
        nc,
        _in_maps(routed_in_egD, w1, w2, w3),
        core_ids=list(range(E)),
        trace=True,
        **trace_kwargs,
    )
    out = np.concatenate([r["out"] for r in res.results], axis=0)
    return out, res


# revision 8
# speedup vs baseline: 1.0299x; 1.0096x over previous
"""Expert-parallel SwiGLU MLP (MoE experts) for 8 Trainium2 NeuronCores.

Problem: routed_in_egD [E*G, D] fp32, w1/w3 [E, D, F], w2 [E, F, D], E=8,
G=2048, D=2048, F=5632.  reference:
    x_egD = routed.reshape(E, G, D)
    mid   = silu(x @ w1) * (x @ w3)          # [E, G, F]
    out   = (mid @ w2).reshape(E*G, D)

Sharding: expert-parallel - core e gets expert e's x slice + weights; no
collectives.  Each core runs three 2048x2048x5632-class GEMMs (~142 GFLOP),
matmul-roofline-bound at the bf16 PE rate (1 col/cycle @ 2.4 GHz -> ~1.80 ms
of pure matmul streaming per core), so the whole game is keeping the PE at
~100% MATMUL occupancy: no PE transposes, no phase-boundary stalls.

Per-core kernel (all matmuls bf16, PSUM fp32):
  phase A: SWDGE casts x -> xbf (DRAM, bf16) in 4 g-blocks; HWDGE xbar
           DMA-transposes each block DRAM->SBUF into its own contiguous tile
           xTb[b] [P, DO, 512] (d = do*128 + p row mapping).  SBUF->SBUF
           transposes are NOT used (Tile's deadlock-avoidance serializes them
           to ~10us each); DRAM-sourced transposes run at full rate.  The PE
           does zero transpose work.
  phase 1: gh-outer (g-halves): per (gh, fc): gateT/upT = w1/w3.T @ x
           accumulated over d in PSUM (j-outer so the first 32 matmuls need
           only xTb[0]); SwiGLU (ACT silu -> bf16, DVE mul); midT spilled to
           DRAM bf16 per-gp tiles => precise DMA deps.  w1/w3 are re-read per
           gh (DMA has headroom; PE does not).  w2 dq=0 panel is cast-loaded
           into a pre-allocated buffer DURING gh=0 (interleaved on the SWDGE
           queue), and mq panels for early gp prefetch on the idle sync queue,
           so phase 2 starts with zero DMA wait.
  phase 2: out[g,d] = sum_f midT[f,g]*w2[f,d]: mid panels stationary (bf16),
           w2 panels DMA-cast fp32->bf16 (moving), PSUM accumulation over F.
"""

import numpy as np

import concourse.mybir as mybir
import concourse.tile as tile
from concourse import bacc
from concourse.bass_utils import run_bass_kernel_spmd

E, G, D, F = 8, 2048, 2048, 5632
P = 128
DO = D // P      # 16 d-chunks (contraction steps)
FC = F // P      # 44 f-chunks
GB = 4           # g-blocks of 512 for the x transpose

F32 = mybir.dt.float32
BF16 = mybir.dt.bfloat16


def build_nc():
    nc = bacc.Bacc("TRN2", target_bir_lowering=False)
    x = nc.dram_tensor("x", [G, D], F32, kind="ExternalInput").ap()
    w1 = nc.dram_tensor("w1", [D, F], F32, kind="ExternalInput").ap()
    w2 = nc.dram_tensor("w2", [F, D], F32, kind="ExternalInput").ap()
    w3 = nc.dram_tensor("w3", [D, F], F32, kind="ExternalInput").ap()
    out = nc.dram_tensor("out", [G, D], F32, kind="ExternalOutput").ap()

    # xbar transpose of xbf[g-block, :] into xTb[b] [p, do, g] maps rows as
    # d = do*128 + p (natural d-chunking: partition p within chunk do).
    w1r = w1.rearrange("(do p) f -> p do f", p=P)
    w3r = w3.rearrange("(do p) f -> p do f", p=P)
    w2r = w2.rearrange("(fo p) d -> p fo d", p=P)

    with tile.TileContext(nc) as tc:
        dram = tc.alloc_tile_pool(name="dram", bufs=1, space="DRAM")
        xbf = dram.tile([G, D], BF16, name="xbf")
        # per-gp mid tiles: mids[gp][p, fo, g'] = mid for f = fo*128+p,
        # g = gp*256 + g'.  Phase-1 writes are per-partition contiguous 512B;
        # phase-2 panel reads are per-partition contiguous 22KB.
        mids = [
            dram.tile([P, FC, 256], BF16, tag=f"mid{gp}", name=f"mid{gp}")
            for gp in range(8)
        ]

        # right-side pools pre-allocated so phase-2 inputs stream in during
        # phase 1 (their SBUF space never overlaps the phase-1 pools)
        mqp = tc.alloc_tile_pool(name="mqp", bufs=3, side="right")
        w2p0 = tc.alloc_tile_pool(name="w2p0", bufs=1, side="right")
        w2q0 = w2p0.tile([P, FC, 512], BF16, name="w2q0")

        xtp = tc.alloc_tile_pool(name="xtp", bufs=1)
        xTb = [
            xtp.tile([P, DO, 512], BF16, tag=f"xTb{b}", name=f"xTb{b}")
            for b in range(GB)
        ]

        # ---- phase A part 1: stage + transpose the first g-half of x
        for b in range(2):
            nc.gpsimd.dma_start(
                xbf[b * 512 : (b + 1) * 512, :], x[b * 512 : (b + 1) * 512, :]
            )
            nc.sync.dma_start_transpose(xTb[b], xbf[b * 512 : (b + 1) * 512, :])

        # ---- phase 1: midT[f, g] = silu(w1.T x) * (w3.T x), spill bf16
        wp = tc.alloc_tile_pool(name="wp", bufs=3)
        mp = tc.alloc_tile_pool(name="mp", bufs=4)
        ps1g = tc.alloc_tile_pool(name="ps1g", bufs=2, space="PSUM")
        ps1u = tc.alloc_tile_pool(name="ps1u", bufs=2, space="PSUM")
        w2bounds = [0, 6, 12, 18, 24, 29, 34, 39, 44]
        for gh in range(2):
            for fc in range(FC):
                w1t = wp.tile([P, DO, P], BF16, tag="w1")
                nc.gpsimd.dma_start(w1t, w1r[:, :, fc * P : (fc + 1) * P])
                w3t = wp.tile([P, DO, P], BF16, tag="w3")
                nc.gpsimd.dma_start(w3t, w3r[:, :, fc * P : (fc + 1) * P])
                if gh == 0 and 1 <= fc <= 2:
                    # remaining x g-blocks: consumed from fc=0 of gh=1 onward
                    b = fc + 1
                    nc.gpsimd.dma_start(
                        xbf[b * 512 : (b + 1) * 512, :],
                        x[b * 512 : (b + 1) * 512, :],
                    )
                    nc.sync.dma_start_transpose(
                        xTb[b], xbf[b * 512 : (b + 1) * 512, :]
                    )
                if gh == 0 and 8 <= fc < 16:
                    # stream the dq=0 w2 panel into its pre-allocated buffer
                    # while the SWDGE queue has slack
                    lo, hi = w2bounds[fc - 8], w2bounds[fc - 7]
                    nc.gpsimd.dma_start(
                        w2q0[:, lo:hi, :], w2r[:, lo:hi, 0:512]
                    )
                pg = ps1g.tile([P, 2, 512], F32, tag="pg")
                pu = ps1u.tile([P, 2, 512], F32, tag="pu")
                for j in range(2):
                    xs = xTb[gh * 2 + j]
                    for d in range(DO):
                        st, sp_ = (d == 0), (d == DO - 1)
                        nc.tensor.matmul(
                            pg[:, j], w1t[:, d], xs[:, d], start=st, stop=sp_
                        )
                        nc.tensor.matmul(
                            pu[:, j], w3t[:, d], xs[:, d], start=st, stop=sp_
                        )
                mo = mp.tile([P, 4, 256], BF16, tag="mo")
                nc.scalar.activation(
                    mo, pg.rearrange("p j g -> p (j g)"),
                    mybir.ActivationFunctionType.Silu,
                )
                nc.vector.tensor_mul(mo, mo, pu.rearrange("p j g -> p (j g)"))
                for j in range(4):
                    nc.scalar.dma_start(mids[gh * 4 + j][:, fc], mo[:, j])
        mp.release()
        wp.release()
        xtp.release()
        ps1u.release()
        ps1g.release()

        # ---- phase 2: out[g, d] = midT.T @ w2 (bf16 x bf16, fp32 psum)
        w2p = tc.alloc_tile_pool(name="w2p", bufs=2, side="right")
        op = tc.alloc_tile_pool(name="op", bufs=4, side="right")
        ps2 = tc.alloc_tile_pool(name="ps2", bufs=3, space="PSUM")
        for dq in range(4):
            if dq == 0:
                w2q = w2q0
            else:
                w2q = w2p.tile([P, FC, 512], BF16, tag="w2q")
                nc.gpsimd.dma_start(w2q, w2r[:, :, dq * 512 : (dq + 1) * 512])
            for gp in range(8):
                mq = mqp.tile([P, FC, 256], BF16, tag="mq")
                dma_eng = nc.sync if gp % 2 == 0 else nc.scalar
                dma_eng.dma_start(mq, mids[gp])
                po = ps2.tile([P, 2, 512], F32, tag="po")
                for fo in range(FC):
                    st, sp_ = (fo == 0), (fo == FC - 1)
                    for gc in range(2):
                        nc.tensor.matmul(
                            po[:, gc],
                            mq[:, fo, gc * P : (gc + 1) * P],
                            w2q[:, fo],
                            start=st,
                            stop=sp_,
                        )
                for gc in range(2):
                    ot = op.tile([P, 512], F32, tag="ot")
                    nc.vector.tensor_copy(ot, po[:, gc])
                    g00 = (gp * 2 + gc) * P
                    nc.sync.dma_start(
                        out[g00 : g00 + P, dq * 512 : (dq + 1) * 512], ot
                    )
        op.release()
        w2p.release()
        ps2.release()
        w2p0.release()
        mqp.release()
        dram.release()
    nc.compile()
    return nc


_NC_CACHE = None


def _get_nc():
    global _NC_CACHE
    if _NC_CACHE is None:
        _NC_CACHE = build_nc()
    return _NC_CACHE


def _in_maps(routed_in_egD, w1, w2, w3):
    x = np.ascontiguousarray(np.asarray(routed_in_egD, dtype=np.float32))
    w1 = np.ascontiguousarray(np.asarray(w1, dtype=np.float32))
    w2 = np.ascontiguousarray(np.asarray(w2, dtype=np.float32))
    w3 = np.ascontiguousarray(np.asarray(w3, dtype=np.float32))
    x_e = x.reshape(E, G, D)
    return [
        {"x": x_e[e], "w1": w1[e], "w2": w2[e], "w3": w3[e]} for e in range(E)
    ]


def kernel(routed_in_egD, w1, w2, w3):
    nc = _get_nc()
    in_maps = _in_maps(routed_in_egD, w1, w2, w3)
    try:
        res = run_bass_kernel_spmd(nc, in_maps, core_ids=list(range(E)))
    except Exception:
        # the first execute after process start occasionally dies with a
        # transient NRT_EXEC_UNIT_UNRECOVERABLE through the PJRT tunnel;
        # a straight retry has always succeeded
        res = run_bass_kernel_spmd(nc, in_maps, core_ids=list(range(E)))
    return np.concatenate([r["out"] for r in res.results], axis=0)


def run_traced(routed_in_egD, w1, w2, w3, **trace_kwargs):
    """For test.py: run with NTFF tracing; returns (full_out, BassKernelResults)."""
    nc = _get_nc()
    res = run_bass_kernel_spmd(
        nc,
        _in_maps(routed_in_egD, w1, w2, w3),
        core_ids=list(range(E)),
        trace=True,
        **trace_kwargs,
    )
    out = np.concatenate([r["out"] for r in res.results], axis=0)
    return out, res
